# revision 8
# baseline (speedup 1.0000x reference)
"""AuxLossFreeGate (DeepSeek-style MoE router gate) for Trainium2, 8 NeuronCores.

Data-parallel over tokens: each of the 8 cores handles 4096 of the 32768
tokens. Per core:
  scores  = sigmoid(x @ W^T)            [4096, 256]
  biased  = scores + bias
  group-limited top-k routing (8 groups of 32; keep top-4 groups by
  sum-of-top-2; top-8 experts over kept groups by biased score)
  weights = 2.5 * normalized raw scores of the selected experts
Outputs (weights [T,8] f32, indices [T,8] int32), slot order = descending
biased score (jax.lax.top_k semantics incl. stable tie-breaking).
"""
import sys
import numpy as np

sys.path.insert(0, "/opt/trn_rl_repo")

import concourse.bass as bass  # noqa: E402
import concourse.mybir as mybir  # noqa: E402
import concourse.tile as tile  # noqa: E402
from concourse import bacc  # noqa: E402
from concourse.bass_utils import run_bass_kernel_spmd  # noqa: E402

F32 = mybir.dt.float32
U32 = mybir.dt.uint32
AF = mybir.ActivationFunctionType
OP = mybir.AluOpType

N_CORES = 8
T_FULL = 32768
T = T_FULL // N_CORES        # 4096 tokens per core
D = 2048
E = 256
N_GROUPS = 8
EPG = E // N_GROUPS          # 32 experts per group
TOPK = 8
NTILES = T // 128            # 32 token tiles per core
KC = D // 128                # 16 contraction chunks
NEG = -1e30
ROUTE_SCALE = 2.5

# float32r: TensorE fp32 fast path (~tf32 operand rounding, full-rate).
# float32: exact but 4x slower on the PE.
import os as _os
MM_DT = (mybir.dt.float32 if _os.environ.get("GATE_MM_DT") == "f32"
         else mybir.dt.float32r)


def build(mm_dt=MM_DT):
    nc = bacc.Bacc("TRN2", target_bir_lowering=False)

    xs = nc.dram_tensor("xs", [T, D], mm_dt, kind="ExternalInput")
    wT = nc.dram_tensor("wT", [D, E], mm_dt, kind="ExternalInput")
    bias_bc = nc.dram_tensor("bias_bc", [128, E], F32, kind="ExternalInput")
    ident = nc.dram_tensor("ident", [128, 128], mm_dt, kind="ExternalInput")
    w_out = nc.dram_tensor("w_out", [T, TOPK], F32, kind="ExternalOutput")
    i_out = nc.dram_tensor("i_out", [T, TOPK], U32, kind="ExternalOutput")

    with tile.TileContext(nc) as tc:
        with (
            tc.tile_pool(name="const", bufs=1) as const,
            tc.tile_pool(name="xload", bufs=3) as xload,
            tc.tile_pool(name="xtsb", bufs=2) as xtsb,
            tc.tile_pool(name="xtps", bufs=2, space="PSUM") as xtps,
            tc.tile_pool(name="lgps", bufs=2, space="PSUM") as lgps,
            tc.tile_pool(name="big", bufs=1) as big,
            tc.tile_pool(name="scr", bufs=2) as scr,
        ):
            # ---- constants / persistent buffers ----
            wT_sb = const.tile([128, KC * E], mm_dt)      # chunk k at cols k*E:(k+1)*E
            nc.sync.dma_start(
                wT_sb[:].rearrange("p (k e) -> p k e", k=KC),
                wT[:].rearrange("(k p) e -> p k e", p=128),
            )
            bias_sb = const.tile([128, E], F32)
            nc.sync.dma_start(bias_sb[:], bias_bc[:])
            id_sb = const.tile([128, 128], mm_dt)
            nc.sync.dma_start(id_sb[:], ident[:])

            scores_all = big.tile([128, NTILES * E], F32)
            m1_all = big.tile([128, NTILES * 8], F32)
            m2_all = big.tile([128, NTILES * 8], F32)
            gs_all = big.tile([128, NTILES * 8], F32)
            gsort_all = big.tile([128, NTILES * 8], F32)
            pen_all = big.tile([128, NTILES * 8], F32)
            vals_all = big.tile([128, NTILES * 8], F32)
            idx_all = big.tile([128, NTILES * 8], U32)
            rawv_all = big.tile([128, NTILES * 8], F32)
            rawi_all = big.tile([128, NTILES * 8], U32)
            s_all = big.tile([128, NTILES], F32)
            recip_all = big.tile([128, NTILES], F32)
            wpre_all = big.tile([128, NTILES * 8], F32)
            w_all = big.tile([128, NTILES * 8], F32)

            for t in range(NTILES):
                # ---- load x tile [128 tokens, D] ----
                x_t = xload.tile([128, D], mm_dt)
                nc.sync.dma_start(x_t[:], xs[t * 128:(t + 1) * 128, :])

                # ---- transpose to x^T chunks, via PE, in 2 halves ----
                xt_t = xtsb.tile([128, D], mm_dt)
                for h in range(2):
                    xt_p = xtps.tile([128, D // 2], mm_dt)
                    for k in range(KC // 2):
                        c = h * (KC // 2) + k
                        nc.tensor.transpose(
                            xt_p[:, k * 128:(k + 1) * 128],
                            x_t[:, c * 128:(c + 1) * 128],
                            id_sb[:],
                        )
                    # PSUM -> SBUF on the scalar engine (f32r out => rounded)
                    nc.scalar.copy(
                        xt_t[:, h * (D // 2):(h + 1) * (D // 2)],
                        xt_p[:],
                    )

                # ---- logits = x @ W^T : accumulate over 16 K-chunks ----
                lg = lgps.tile([128, E], F32)
                for k in range(KC):
                    nc.tensor.matmul(
                        lg[:],
                        xt_t[:, k * 128:(k + 1) * 128],
                        wT_sb[:, k * E:(k + 1) * E],
                        start=(k == 0),
                        stop=(k == KC - 1),
                    )

                # ---- scores = sigmoid(logits) ----
                sc = scores_all[:, t * E:(t + 1) * E]
                nc.scalar.activation(sc, lg[:], AF.Sigmoid)

                # ---- biased = scores + bias ----
                biased = scr.tile([128, E], F32, tag="biased")
                nc.vector.tensor_tensor(biased[:], sc, bias_sb[:], OP.add)

                # ---- group scores: m1 + m2 (top-2 of each group of 32) ----
                m1 = m1_all[:, t * 8:(t + 1) * 8]
                nc.vector.tensor_reduce(
                    m1, biased[:].rearrange("p (g j) -> p g j", g=N_GROUPS),
                    axis=mybir.AxisListType.X, op=OP.max,
                )
                mr = scr.tile([128, E], F32, tag="mr")
                nc.vector.match_replace(mr[:], m1, biased[:], NEG)
                m2 = m2_all[:, t * 8:(t + 1) * 8]
                nc.vector.tensor_reduce(
                    m2, mr[:].rearrange("p (g j) -> p g j", g=N_GROUPS),
                    axis=mybir.AxisListType.X, op=OP.max,
                )
                gs = gs_all[:, t * 8:(t + 1) * 8]
                nc.vector.tensor_tensor(gs, m1, m2, OP.add)

                # ---- top-4 groups: penalty = (gs < 4th-largest) * NEG ----
                gso = gsort_all[:, t * 8:(t + 1) * 8]
                nc.vector.max(gso, gs)
                pen = pen_all[:, t * 8:(t + 1) * 8]
                nc.vector.tensor_scalar(
                    pen, gs, gsort_all[:, t * 8 + 3:t * 8 + 4], NEG,
                    op0=OP.is_lt, op1=OP.mult,
                )

                # ---- penalized biased scores ----
                pena = scr.tile([128, E], F32, tag="pena")
                nc.vector.tensor_tensor(
                    pena[:].rearrange("p (g j) -> p g j", g=N_GROUPS),
                    biased[:].rearrange("p (g j) -> p g j", g=N_GROUPS),
                    pen.broadcast_to([128, N_GROUPS, EPG]),
                    OP.add,
                )

                # ---- top-8 experts by penalized-biased score ----
                vals = vals_all[:, t * 8:(t + 1) * 8]
                nc.vector.max(vals, pena[:])
                idx = idx_all[:, t * 8:(t + 1) * 8]
                nc.vector.max_index(idx, vals, pena[:])

                # ---- raw scores of the selected experts (+ their sum) ----
                rs = scr.tile([128, E], F32, tag="rs")
                nc.vector.scalar_tensor_tensor(
                    rs[:], pena[:], vals_all[:, t * 8 + 7:t * 8 + 8], sc,
                    op0=OP.is_ge, op1=OP.mult,
                    accum_out=s_all[:, t:t + 1],
                )
                rawv = rawv_all[:, t * 8:(t + 1) * 8]
                nc.vector.max(rawv, rs[:])
                rawi = rawi_all[:, t * 8:(t + 1) * 8]
                nc.vector.max_index(rawi, rawv, rs[:])

            # ---- normalization factors: 2.5 / S ----
            nc.vector.reciprocal(recip_all[:], s_all[:])
            nc.vector.tensor_scalar(
                recip_all[:], recip_all[:], ROUTE_SCALE, None, op0=OP.mult,
            )

            # ---- reorder raw scores into biased-rank slots, batched 4 tiles ----
            GB = 4  # tiles per batch
            for g in range(NTILES // GB):
                sl = slice(g * GB * 8, (g + 1) * GB * 8)
                idx_b = idx_all[:, sl].rearrange("p (t k) -> p t k", t=GB)
                rawi_b = rawi_all[:, sl].rearrange("p (t k) -> p t k", t=GB)
                rawv_b = rawv_all[:, sl].rearrange("p (t k) -> p t k", t=GB)
                eq = scr.tile([128, GB * 8 * 8], F32, tag="eq")
                eq_v = eq[:].rearrange("p (t k j) -> p t k j", t=GB, k=8)
                nc.vector.tensor_tensor(
                    eq_v,
                    idx_b.unsqueeze(3).broadcast_to([128, GB, 8, 8]),
                    rawi_b.unsqueeze(2).broadcast_to([128, GB, 8, 8]),
                    OP.is_equal,
                )
                w8 = scr.tile([128, GB * 8 * 8], F32, tag="w8")
                w8_v = w8[:].rearrange("p (t k j) -> p t k j", t=GB, k=8)
                nc.vector.tensor_tensor(
                    w8_v, eq_v,
                    rawv_b.unsqueeze(2).broadcast_to([128, GB, 8, 8]),
                    OP.mult,
                )
                nc.vector.tensor_reduce(
                    wpre_all[:, sl].rearrange("p (t k) -> p t k", t=GB),
                    w8_v, axis=mybir.AxisListType.X, op=OP.add,
                )
                nc.vector.scalar_tensor_tensor(
                    w_all[:, sl].rearrange("p (t k) -> p t k", t=GB),
                    wpre_all[:, sl].rearrange("p (t k) -> p t k", t=GB),
                    1.0,
                    recip_all[:, g * GB:(g + 1) * GB]
                    .unsqueeze(2).broadcast_to([128, GB, 8]),
                    op0=OP.mult, op1=OP.mult,
                )

            # ---- store outputs ----
            nc.sync.dma_start(
                w_out[:].rearrange("(t p) k -> p t k", p=128),
                w_all[:].rearrange("p (t k) -> p t k", t=NTILES),
            )
            nc.sync.dma_start(
                i_out[:].rearrange("(t p) k -> p t k", p=128),
                idx_all[:].rearrange("p (t k) -> p t k", t=NTILES),
            )

    nc.finalize()
    return nc


_CACHE = {}


def _get_nc():
    if "nc" not in _CACHE:
        _CACHE["nc"] = build(MM_DT)
    return _CACHE["nc"]


def kernel(x, weight, bias, _trace=False):
    x = np.ascontiguousarray(x, dtype=np.float32)
    weight = np.ascontiguousarray(weight, dtype=np.float32)
    bias = np.ascontiguousarray(bias, dtype=np.float32)

    wT = np.ascontiguousarray(weight.T)                     # [D, E]
    bias_bc = np.ascontiguousarray(np.broadcast_to(bias[None, :], (128, E)))
    ident = np.eye(128, dtype=np.float32)

    in_maps = []
    for c in range(N_CORES):
        in_maps.append({
            "xs": x[c * T:(c + 1) * T],
            "wT": wT,
            "bias_bc": bias_bc,
            "ident": ident,
        })

    nc = _get_nc()
    res = run_bass_kernel_spmd(nc, in_maps, core_ids=list(range(N_CORES)),
                               trace=_trace)
    weights = np.concatenate([r["w_out"] for r in res.results], axis=0)
    indices = np.concatenate([r["i_out"] for r in res.results], axis=0)
    out = (weights.astype(np.float32), indices.astype(np.int32))
    if _trace:
        return out, res
    return out
